# revision 3
# baseline (speedup 1.0000x reference)
"""Bilinear causal attention (nn_Attention_34772055228779) on 8 trn2 cores.

reference:
  scores[i,k] = x[i] @ W_bi[k] @ x[i]          [512, 512]
  attn = softmax(scores + causal_mask, axis=1)
  out  = (attn @ x) @ W_out.T                  [512, 512]

Device strategy (tensor-parallel over score columns, per sharding hint):
  core m holds W_bi[64m:64(m+1)]  (64 MiB fp32)
  stage A: for each local k: Y_k = X @ W_k  (fp32r matmuls, lhsT = X^T resident)
           scores[:, k] = rowsum(Y_k * X)   (fused DVE scalar_tensor_tensor)
  AllToAll over the [8 x 64-row, 64-col] score shard blocks: core m ends up
           with rows [64m, 64m+64) of the FULL score matrix.
  tail:    masked softmax rows (ACT exp with fused accum), A^T via PE
           transpose, O^T = X^T A^T, Y = O @ W_out^T, DMA 64 output rows.
  host:    concatenates the 8 row blocks.
"""
import numpy as np

N_CTX = 512
D = 512
NCORES = 8
KSH = N_CTX // NCORES      # 64 score columns per core
RSH = N_CTX // NCORES      # 64 output rows per core
NEG_INF = -1e30

_nc_cache = None


def _build():
    import concourse.mybir as mybir
    import concourse.tile as tile
    from concourse import bacc

    f32 = mybir.dt.float32
    f32r = mybir.dt.float32r
    Alu = mybir.AluOpType
    Act = mybir.ActivationFunctionType

    nc = bacc.Bacc(
        "TRN2", target_bir_lowering=False, debug=False,
        enable_asserts=False, num_devices=NCORES,
    )

    x_t = nc.dram_tensor("x", [N_CTX, D], f32, kind="ExternalInput").ap()
    xt_t = nc.dram_tensor("xt", [D, N_CTX], f32, kind="ExternalInput").ap()
    wbi_t = nc.dram_tensor("wbi", [KSH, D, D], f32, kind="ExternalInput").ap()
    woutt_t = nc.dram_tensor("wout_t", [D, D], f32, kind="ExternalInput").ap()
    mask_t = nc.dram_tensor("mask", [RSH, N_CTX], f32, kind="ExternalInput").ap()
    ident_t = nc.dram_tensor("ident", [128, 128], f32, kind="ExternalInput").ap()
    out_t = nc.dram_tensor("out", [RSH, D], f32, kind="ExternalOutput").ap()

    with tile.TileContext(nc) as tc:
        with (
            tc.tile_pool(name="const", bufs=1) as cpool,
            tc.tile_pool(name="wstream", bufs=4) as wpool,
            tc.tile_pool(name="scratch", bufs=3) as spool,
            tc.tile_pool(name="small", bufs=1) as mpool,
            tc.tile_pool(name="psA", bufs=6, space="PSUM") as ppA,
            tc.tile_pool(name="psB", bufs=2, space="PSUM") as ppB,
            tc.tile_pool(name="dram", bufs=1, space="DRAM") as dpool,
        ):
            # ---- resident loads -------------------------------------------
            x_sb, xt_sb, woutt_sb = [], [], []
            for t in range(4):
                a = cpool.tile([128, N_CTX], f32, tag=f"x{t}", name=f"x{t}")
                nc.sync.dma_start(a[:], x_t[t * 128:(t + 1) * 128, :])
                x_sb.append(a)
                b = cpool.tile([128, N_CTX], f32r, tag=f"xt{t}", name=f"xt{t}")
                nc.sync.dma_start(b[:], xt_t[t * 128:(t + 1) * 128, :].bitcast(f32r))
                xt_sb.append(b)
                c = cpool.tile([128, D], f32, tag=f"wo{t}", name=f"wo{t}")
                nc.sync.dma_start(c[:], woutt_t[t * 128:(t + 1) * 128, :])
                woutt_sb.append(c)
            mask_sb = cpool.tile([RSH, N_CTX], f32, tag="mask")
            nc.sync.dma_start(mask_sb[:], mask_t[:])
            ident_sb = cpool.tile([128, 128], f32, tag="ident")
            nc.sync.dma_start(ident_sb[:], ident_t[:])
            scores_sb = [
                cpool.tile([128, KSH], f32, tag=f"sc{t}", name=f"sc{t}")
                for t in range(4)
            ]

            # ---- stage A: local score columns -----------------------------
            for kk in range(KSH):
                wk = wpool.tile([128, 4, D], f32r, tag="wk")
                nc.sync.dma_start(
                    wk[:],
                    wbi_t[kk].rearrange("(dt p) e -> p dt e", p=128).bitcast(f32r),
                )
                for nt in range(4):
                    yp = ppA.tile([128, D], f32, tag="yp")
                    for dt in range(4):
                        nc.tensor.matmul(
                            yp[:],
                            lhsT=xt_sb[dt][:, nt * 128:(nt + 1) * 128],
                            rhs=wk[:, dt, :],
                            start=(dt == 0),
                            stop=(dt == 3),
                        )
                    scr = spool.tile([128, D], f32, tag="stt_out")
                    nc.vector.scalar_tensor_tensor(
                        out=scr[:], in0=yp[:], scalar=1.0, in1=x_sb[nt][:],
                        op0=Alu.mult, op1=Alu.mult,
                        accum_out=scores_sb[nt][:, kk:kk + 1],
                    )

            # ---- AllToAll: shard columns -> shard rows --------------------
            agin = dpool.tile([N_CTX, KSH], f32, tag="agin")
            agout = dpool.tile([N_CTX, KSH], f32, tag="agout")
            for nt in range(4):
                nc.sync.dma_start(agin[nt * 128:(nt + 1) * 128, :], scores_sb[nt][:])
            nc.gpsimd.collective_compute(
                "AllToAll",
                mybir.AluOpType.bypass,
                replica_groups=[list(range(NCORES))],
                ins=[agin[:].opt()],
                outs=[agout[:].opt()],
            )
            # rows of the full score matrix for this core: [64, 512]
            sfull = mpool.tile([RSH, N_CTX], f32, tag="sfull")
            nc.sync.dma_start(
                sfull[:].rearrange("i (r k) -> i r k", r=NCORES),
                agout[:].rearrange("(r i) k -> i r k", r=NCORES),
            )

            # ---- masked softmax over the 64 rows --------------------------
            sm = mpool.tile([RSH, N_CTX], f32, tag="sm")
            nc.vector.tensor_tensor(
                out=sm[:], in0=sfull[:], in1=mask_sb[:], op=Alu.add)
            negm = mpool.tile([RSH, 1], f32, tag="negm")
            nc.vector.reduce_max(negm[:], sm[:], axis=mybir.AxisListType.X,
                                 negate=True)
            esb = mpool.tile([RSH, N_CTX], f32, tag="esb")
            den = mpool.tile([RSH, 1], f32, tag="den")
            nc.scalar.activation(
                esb[:], sm[:], Act.Exp, bias=negm[:], scale=1.0, accum_out=den[:])
            rden = mpool.tile([RSH, 1], f32, tag="rden")
            nc.vector.reciprocal(rden[:], den[:])
            a_sb = mpool.tile([RSH, N_CTX], f32, tag="a_sb")
            nc.vector.tensor_scalar_mul(a_sb[:], esb[:], rden[:])

            # ---- A^T via PE transpose: [64, 512] -> 4x [128, 64] ----------
            at_sb = []
            for kt in range(4):
                tp = ppB.tile([128, 512], f32, tag="tail")
                nc.tensor.transpose(
                    tp[:, 0:RSH],
                    a_sb[:, kt * 128:(kt + 1) * 128],
                    ident_sb[0:RSH, 0:RSH],
                )
                at = mpool.tile([128, RSH], f32, tag=f"at{kt}", name=f"at{kt}")
                nc.scalar.copy(at[:], tp[:, 0:RSH])
                at_sb.append(at)

            # ---- O^T = X^T @ A^T : [512(e), 64(i)] ------------------------
            ot_sb = []
            for et in range(4):
                op = ppB.tile([128, 512], f32, tag="tail")
                for kt in range(4):
                    nc.tensor.matmul(
                        op[:, 0:RSH],
                        lhsT=x_sb[kt][:, et * 128:(et + 1) * 128],
                        rhs=at_sb[kt][:],
                        start=(kt == 0),
                        stop=(kt == 3),
                    )
                ot = mpool.tile([128, RSH], f32, tag=f"ot{et}", name=f"ot{et}")
                nc.scalar.copy(ot[:], op[:, 0:RSH])
                ot_sb.append(ot)

            # ---- Y = O @ W_out^T : [64(i), 512(f)] ------------------------
            ypz = ppB.tile([128, 512], f32, tag="tail")
            for et in range(4):
                nc.tensor.matmul(
                    ypz[0:RSH, :],
                    lhsT=ot_sb[et][:],
                    rhs=woutt_sb[et][:],
                    start=(et == 0),
                    stop=(et == 3),
                )
            y_sb = mpool.tile([RSH, D], f32, tag="y_sb")
            nc.scalar.copy(y_sb[:], ypz[0:RSH, :])
            nc.sync.dma_start(out_t[:], y_sb[:])

    nc.compile()
    return nc


def _make_in_maps(x, W_bi, W_out):
    x = np.ascontiguousarray(np.asarray(x, dtype=np.float32))
    W_bi = np.asarray(W_bi, dtype=np.float32)
    W_out = np.asarray(W_out, dtype=np.float32)
    xt = np.ascontiguousarray(x.T)
    woutt = np.ascontiguousarray(W_out.T)
    mask_full = np.triu(np.full((N_CTX, N_CTX), NEG_INF, dtype=np.float32), 1)
    ident = np.eye(128, dtype=np.float32)
    in_maps = []
    for m in range(NCORES):
        in_maps.append({
            "x": x,
            "xt": xt,
            "wbi": np.ascontiguousarray(W_bi[m * KSH:(m + 1) * KSH]),
            "wout_t": woutt,
            "mask": np.ascontiguousarray(mask_full[m * RSH:(m + 1) * RSH]),
            "ident": ident,
        })
    return in_maps


def kernel(x, W_bi, W_out):
    global _nc_cache
    from concourse.bass_utils import run_bass_kernel_spmd

    if _nc_cache is None:
        _nc_cache = _build()
    nc = _nc_cache
    in_maps = _make_in_maps(x, W_bi, W_out)
    res = run_bass_kernel_spmd(nc, in_maps, core_ids=list(range(NCORES)),
                               trace=False)
    out = np.concatenate([res.results[m]["out"] for m in range(NCORES)], axis=0)
    return np.ascontiguousarray(out, dtype=np.float32)
